# revision 55
# baseline (speedup 1.0000x reference)
"""Dense SE(3) Gauss-Newton kernel for Trainium2, sharded over 8 NeuronCores.

Sharding: core owns batch b = core//4 and a 256-anchor slab of the i axis;
the k axis (1024) runs in 8 chunks of 128 on the partition dimension with
anchors on the free dimension.

Per (i,k) the kernel materializes 13 fp16 "moving bands" (powers of the
projected-point deltas d, dX, dY, optionally weighted by the embedding
affinity); the 6x6 normal equations + rhs are accumulated straight into
PSUM as 27 entry rows by matmuls against host-precomputed per-k fp16
coefficient tables (per-entry stationaries).  The residual is decomposed
around each point's self-projection (delta form) so every band is
cancellation-free and fp16-safe.  The geometry inputs are pre-quantized
to fp16 on the host and every host-side constant (self-projections,
coefficient tables) is derived from the quantized values, which makes a
single-pass fp16 geometry matmul exact-enough by construction.  1/Z runs
on the Vector engine (fast custom reciprocal); the affinity
exp(-sqrt(s)) is batched once so only two ACT table switches happen.
"""
import sys

sys.path.insert(0, "/opt/trn_rl_repo")

import numpy as np

from concourse import bacc, tile
import concourse.mybir as mybir
from concourse.bass_utils import run_bass_kernel_spmd

F32 = mybir.dt.float32
F16 = mybir.dt.float16
AF = mybir.ActivationFunctionType
ALU = mybir.AluOpType
AX = mybir.AxisListType

B, C, H, W = 2, 16, 32, 32
N = H * W
NCORES = 8
SLAB = 256
KC = 8
P = 128
GR = 32                       # geometry contraction rows
HTRI = [(p, q) for p in range(6) for q in range(p, 6)]  # 21 entries
NHB = 6                       # Hm bands: A, AdX, AdY, AdX2, AdY2, Ad2
NRB = 5                       # rhs bands: ddX, ddY, ddX2, ddY2, d2dD
ACC_CW = NHB * 21 + NRB * 6   # 156 stationary cols per chunk
NE = 48                       # augmented 6x7 system padded
FY2_OVER_FX2 = 1.0            # camera intrinsics: fx = fy = W
INV_FX2 = 1.0 / (32.0 * 32.0)


def build_nc():
    nc = bacc.Bacc("TRN2", target_bir_lowering=False, debug=False)

    geom_d = nc.dram_tensor("geom", [GR, 2048], F16, kind="ExternalInput")
    accst_d = nc.dram_tensor("accst", [P, KC * ACC_CW], F16, kind="ExternalInput")
    misc_d = nc.dram_tensor("misc", [P, 144], F32, kind="ExternalInput")
    out_d = nc.dram_tensor("out", [P, 32], F32, kind="ExternalOutput")

    with tile.TileContext(nc) as tc:
        with tc.tile_pool(name="persist", bufs=1) as pp, \
             tc.tile_pool(name="acc_ps", bufs=1, space="PSUM") as accp:

            geom = pp.tile([GR, 2048], F16)
            accst = pp.tile([P, KC * ACC_CW], F16)
            misc = pp.tile([P, 144], F32)
            nc.sync.dma_start(geom[:, 1024:2048], geom_d[:, 1024:2048])
            nc.sync.dma_start(geom[:, 0:256], geom_d[:, 0:256])
            nc.sync.dma_start(geom[:, 256:1024], geom_d[:, 256:1024])
            nc.sync.dma_start(accst[:, 0 : 4 * ACC_CW], accst_d[:, 0 : 4 * ACC_CW])
            nc.sync.dma_start(accst[:, 4 * ACC_CW :], accst_d[:, 4 * ACC_CW :])
            nc.sync.dma_start(misc[:], misc_d[:])
            stat = geom[:, 0:1024]
            mov = geom[:, 1024:2048]
            dkk = misc[:, 0:8]
            tmi0 = misc[:, 8:24]
            tmi1 = misc[:, 24:40]
            EXPh = misc[:, 40:88]          # rows 0:21: Hm-entry -> 6x7 expander
            EXPr = misc[:, 88:136]         # rows 0:6: rhs-entry -> col 6
            nr2f = misc[:, 136:144]        # -r2(k)/fx^2 per chunk col

            lnbias = pp.tile([P, 1], F32)
            nc.vector.memset(lnbias[:], 1e-12)
            sallh = pp.tile([P, 2048], F16)   # ||e_i-e_k||^2, relu'd
            affh = pp.tile([P, 2048], F16)    # exp(-||e_i-e_k||)
            atmp = pp.tile([P, 2048], F32)
            dall = pp.tile([P, 2048], F32)    # d = 1/Zp
            dhall = pp.tile([P, 2048], F16)
            d2all = pp.tile([P, 2048], F16)
            dXYall = pp.tile([P, 4096], F16)  # [dX | dY] per chunk, 512 wide

            # two accumulators per target, each on its own PSUM bank, so
            # consecutive accumulating matmuls never hit the same bank
            accHt = [accp.tile([21, 2 * SLAB], F32, name=f"accH{j}") for j in range(2)]
            accRt = [accp.tile([6, 2 * SLAB], F32, name=f"accR{j}") for j in range(2)]
            accH = [a[:, 0:SLAB] for a in accHt]
            accR = [a[:, 0:SLAB] for a in accRt]

            with tc.tile_pool(name="mm_ps", bufs=2, space="PSUM") as mmp, \
                 tc.tile_pool(name="work", bufs=3) as wp:

                def pass_a(c):
                    ck = slice(c * P, (c + 1) * P)
                    cs = slice(c * SLAB, (c + 1) * SLAB)
                    c2 = slice(c * 2 * SLAB, (c + 1) * 2 * SLAB)
                    Zs = mmp.tile([P, 2 * SLAB], F32, name=f"Zs{c}", tag="Zs")
                    XY = mmp.tile([P, 2 * SLAB], F32, name=f"XY{c}", tag="XY")
                    nc.tensor.matmul(Zs[:], stat[:, ck], mov[:, 512:1024],
                                     start=True, stop=True)
                    nc.tensor.matmul(XY[:], stat[:, ck], mov[:, 0:512],
                                     start=True, stop=True)
                    d32 = dall[:, cs]
                    nc.vector.reciprocal_approx_fast(d32, Zs[:, 0:SLAB])
                    nc.scalar.activation(sallh[:, cs], Zs[:, SLAB : 2 * SLAB], AF.Relu)
                    nc.vector.tensor_tensor(
                        dXYall[:, c2].rearrange("p (b n) -> p b n", b=2),
                        XY[:].rearrange("p (b n) -> p b n", b=2),
                        d32.unsqueeze(1).to_broadcast((P, 2, SLAB)), ALU.mult)
                    nc.scalar.copy(dhall[:, cs], d32)
                    nc.scalar.square(d2all[:, cs], d32)

                def pass_b1(c):
                    # rhs bands (affinity-free): d, ddX, ddY, M3 (merged)
                    cs = slice(c * SLAB, (c + 1) * SLAB)
                    c2 = slice(c * 2 * SLAB, (c + 1) * 2 * SLAB)
                    ro = c * ACC_CW + NHB * 21
                    dXY = dXYall[:, c2]
                    dh = dhall[:, cs]
                    d2h = d2all[:, cs]

                    def wt(nm, w=SLAB):
                        return wp.tile([P, w], F16, name=f"{nm}{c}", tag=nm)

                    bdXY = wt("bdXY", 2 * SLAB)
                    bdXY2 = wt("bdXY2", 2 * SLAB)
                    bd2dD = wt("bd2dD")
                    nc.vector.tensor_tensor(
                        bdXY[:].rearrange("p (b n) -> p b n", b=2),
                        dXY.rearrange("p (b n) -> p b n", b=2),
                        dh.unsqueeze(1).to_broadcast((P, 2, SLAB)), ALU.mult)
                    nc.gpsimd.tensor_tensor(bdXY2[:], bdXY[:], dXY, ALU.mult)
                    nc.vector.scalar_tensor_tensor(bd2dD[:], dall[:, cs],
                                                   dkk[:, c : c + 1],
                                                   d2h, ALU.subtract, ALU.mult)
                    # matmuls in band-availability order; m indexes the
                    # stationary layout [ddX, ddY, ddX2, ddY2, d2dD]
                    rbands = [(bdXY[:, 0:SLAB], 0), (bdXY[:, SLAB:], 1),
                              (bd2dD[:], 4),
                              (bdXY2[:, 0:SLAB], 2), (bdXY2[:, SLAB:], 3)]
                    for j, (bt, m) in enumerate(rbands):
                        nc.tensor.matmul(
                            accR[j % 2], accst[:, ro + m * 6 : ro + (m + 1) * 6], bt,
                            start=(c == 0 and j < 2),
                            stop=(c == KC - 1 and j >= NRB - 2))

                def aff_batch(h):
                    hs = slice(h * 4 * SLAB, (h + 1) * 4 * SLAB)
                    at = atmp[:, hs]
                    nc.scalar.activation(at, sallh[:, hs], AF.Ln, bias=lnbias[:])
                    nc.scalar.activation(at, at, AF.Exp, scale=0.5)
                    nc.scalar.activation(affh[:, hs], at, AF.Exp, scale=-1.0)

                def pass_b2(c):
                    # affinity-weighted Hm bands: A, AdX, AdY, AdX2, AdY2, Ad2
                    cs = slice(c * SLAB, (c + 1) * SLAB)
                    c2 = slice(c * 2 * SLAB, (c + 1) * 2 * SLAB)
                    co = c * ACC_CW
                    dXY = dXYall[:, c2]
                    d2h = d2all[:, cs]

                    def wt(nm, w=SLAB):
                        return wp.tile([P, w], F16, name=f"{nm}{c}", tag=nm)

                    bA = wt("bA")
                    bAdXY = wt("bAdXY", 2 * SLAB)
                    bAdXY2 = wt("bAdXY2", 2 * SLAB)
                    bAd2 = wt("bAd2")
                    nc.vector.tensor_tensor(bA[:], affh[:, cs], d2h, ALU.mult)
                    nc.gpsimd.tensor_tensor(bAd2[:], bA[:], d2h, ALU.mult)
                    nc.vector.tensor_tensor(
                        bAdXY[:].rearrange("p (b n) -> p b n", b=2),
                        dXY.rearrange("p (b n) -> p b n", b=2),
                        bA[:].unsqueeze(1).to_broadcast((P, 2, SLAB)), ALU.mult)
                    nc.vector.tensor_tensor(bAdXY2[:], bAdXY[:], dXY, ALU.mult)
                    hbands = [bA[:], bAdXY[:, 0:SLAB], bAdXY[:, SLAB:],
                              bAdXY2[:, 0:SLAB], bAdXY2[:, SLAB:], bAd2[:]]
                    for m, bt in enumerate(hbands):
                        nc.tensor.matmul(
                            accH[m % 2], accst[:, co + m * 21 : co + (m + 1) * 21], bt,
                            start=(c == 0 and m < 2),
                            stop=(c == KC - 1 and m >= NHB - 2))

                # pass_a runs one chunk ahead so the PE queue always has
                # geometry matmuls to chew on while bands are produced
                pass_a(0)
                for c in range(1, 4):
                    pass_a(c)
                    pass_b1(c - 1)
                aff_batch(0)
                for c in range(4, KC):
                    pass_a(c)
                    pass_b1(c - 1)
                pass_b1(KC - 1)
                for c in range(4):
                    pass_b2(c)
                aff_batch(1)
                for c in range(4, KC):
                    pass_b2(c)

            # ---------------- solve / exp map / compose -----------------
            with tc.tile_pool(name="post", bufs=2) as qp, \
                 tc.tile_pool(name="post_ps", bufs=2, space="PSUM") as qps:
                acc_sbH = qp.tile([21, SLAB], F32)
                acc_sbR = qp.tile([6, SLAB], F32)
                nc.scalar.copy(acc_sbH[:], accH[0])
                nc.scalar.copy(acc_sbR[:], accR[0])
                nc.vector.tensor_tensor(acc_sbH[:], acc_sbH[:], accH[1], ALU.add)
                nc.vector.tensor_tensor(acc_sbR[:], acc_sbR[:], accR[1], ALU.add)

                # expand 27 entry rows -> [anchor, 6x7 augmented] per half
                hb = qp.tile([P, 2 * NE], F32)  # ih-major: [0:48]=ih0, [48:96]=ih1
                for ih in range(2):
                    hb_ps = qps.tile([P, NE], F32, name=f"hbps{ih}", tag="hbps")
                    nc.tensor.matmul(hb_ps[:], acc_sbH[:, ih * P : (ih + 1) * P],
                                     EXPh[0:21, :], start=True, stop=False)
                    nc.tensor.matmul(hb_ps[:], acc_sbR[:, ih * P : (ih + 1) * P],
                                     EXPr[0:6, :], start=False, stop=True)
                    nc.scalar.copy(hb[:, ih * NE : (ih + 1) * NE], hb_ps[:])

                # ---------------- Gauss-Jordan (both halves packed) --------
                def hbv(sl):
                    return hb[:].rearrange("p (i e) -> p i e", i=2)[:, :, sl]
                piv = qp.tile([P, 2], F32)
                f12 = qp.tile([P, 12], F32)
                upd = qp.tile([P, 84], F32)
                f12v = f12[:].rearrange("p (i r) -> p i r", i=2)
                updv = upd[:].rearrange("p (i r c) -> p i r c", r=6, c=7)
                for j in range(6):
                    nc.vector.reciprocal(piv[:], hb[:, 8 * j : 2 * NE : NE])
                    nc.vector.tensor_tensor(
                        f12v, hbv(slice(j, 42, 7)),
                        piv[:].to_broadcast((P, 2, 6)), ALU.mult)
                    nc.vector.memset(f12[:, j : 12 : 6], 0.0)
                    nc.vector.tensor_tensor(
                        updv, f12v.to_broadcast((P, 2, 6, 7)),
                        hbv(slice(7 * j, 7 * j + 7)).unsqueeze(2).to_broadcast((P, 2, 6, 7)),
                        ALU.mult)
                    hview = hbv(slice(0, 42)).rearrange("p i (r c) -> p i r c", c=7)
                    nc.vector.tensor_tensor(hview, hview, updv, ALU.subtract)
                dinv = qp.tile([P, 12], F32)
                delta = qp.tile([P, 12], F32)
                dinvv = dinv[:].rearrange("p (i r) -> p i r", i=2)
                deltav = delta[:].rearrange("p (i r) -> p i r", i=2)
                nc.vector.reciprocal(dinvv, hbv(slice(0, 42, 8)))
                nc.vector.tensor_tensor(deltav, hbv(slice(6, 42, 7)), dinvv, ALU.mult)

                # ------------- exp map coefficients via Taylor in th^2 -----
                wsq = qp.tile([P, 6], F32)
                th2 = qp.tile([P, 2], F32)
                wv = deltav[:, :, 3:6]
                vb = deltav[:, :, 0:3]
                wsqv = wsq[:].rearrange("p (i r) -> p i r", i=2)
                nc.vector.tensor_tensor(wsqv, wv, wv, ALU.mult)
                nc.vector.tensor_reduce(th2[:], wsqv, AX.X, ALU.add)
                tu2 = qp.tile([P, 2], F32)
                tu3 = qp.tile([P, 2], F32)
                nc.vector.tensor_tensor(tu2[:], th2[:], th2[:], ALU.mult)
                nc.vector.tensor_tensor(tu3[:], tu2[:], th2[:], ALU.mult)
                abc = qp.tile([P, 6], F32)   # col = coeff(A,B,C)*2 + ih
                t6 = qp.tile([P, 6], F32)
                # A = sin(t)/t, B = (1-cos t)/t^2, C = (t - sin t)/t^3 series
                nc.vector.tensor_scalar(t6[:, 0:2], th2[:], -1.0 / 6.0, 1.0, ALU.mult, ALU.add)
                nc.vector.tensor_scalar(t6[:, 2:4], th2[:], -1.0 / 24.0, 0.5, ALU.mult, ALU.add)
                nc.vector.tensor_scalar(t6[:, 4:6], th2[:], -1.0 / 120.0, 1.0 / 6.0, ALU.mult, ALU.add)
                nc.vector.scalar_tensor_tensor(abc[:, 0:2], tu2[:], 1.0 / 120.0,
                                               t6[:, 0:2], ALU.mult, ALU.add)
                nc.vector.scalar_tensor_tensor(abc[:, 2:4], tu2[:], 1.0 / 720.0,
                                               t6[:, 2:4], ALU.mult, ALU.add)
                nc.vector.scalar_tensor_tensor(abc[:, 4:6], tu2[:], 1.0 / 5040.0,
                                               t6[:, 4:6], ALU.mult, ALU.add)
                nc.vector.scalar_tensor_tensor(abc[:, 0:2], tu3[:], -1.0 / 5040.0,
                                               abc[:, 0:2], ALU.mult, ALU.add)
                nc.vector.scalar_tensor_tensor(abc[:, 2:4], tu3[:], -1.0 / 40320.0,
                                               abc[:, 2:4], ALU.mult, ALU.add)
                nc.vector.scalar_tensor_tensor(abc[:, 4:6], tu3[:], -1.0 / 362880.0,
                                               abc[:, 4:6], ALU.mult, ALU.add)

                # ------- packed both-half R/V, translation, compose --------
                def iv(tile_ap, n):
                    return tile_ap.rearrange("p (i e) -> p i e", i=n)
                u3 = qp.tile([P, 6], F32)       # (ih, r): w_r^2 - th^2
                u3v = iv(u3[:], 2)
                nc.vector.tensor_tensor(
                    u3v, wsqv, th2[:].unsqueeze(2).to_broadcast((P, 2, 3)),
                    ALU.subtract)
                Aw = qp.tile([P, 6], F32)
                Bw = qp.tile([P, 6], F32)
                Cw = qp.tile([P, 6], F32)
                dB = qp.tile([P, 6], F32)
                dC = qp.tile([P, 6], F32)
                nc.vector.tensor_tensor(
                    iv(Aw[:], 2), wv,
                    abc[:, 0:2].unsqueeze(2).to_broadcast((P, 2, 3)), ALU.mult)
                nc.vector.tensor_tensor(
                    iv(Bw[:], 2), wv,
                    abc[:, 2:4].unsqueeze(2).to_broadcast((P, 2, 3)), ALU.mult)
                nc.vector.tensor_tensor(
                    iv(Cw[:], 2), wv,
                    abc[:, 4:6].unsqueeze(2).to_broadcast((P, 2, 3)), ALU.mult)
                nc.vector.tensor_tensor(
                    iv(dB[:], 2), u3v,
                    abc[:, 2:4].unsqueeze(2).to_broadcast((P, 2, 3)), ALU.mult)
                nc.vector.tensor_tensor(
                    iv(dC[:], 2), u3v,
                    abc[:, 4:6].unsqueeze(2).to_broadcast((P, 2, 3)), ALU.mult)

                def wcol(r):
                    return delta[:, 3 + r : 12 : 6]
                qb = qp.tile([P, 6], F32)   # q01,q02,q12 x (2 ih): col=q*2+ih
                cb = qp.tile([P, 6], F32)
                nc.vector.tensor_tensor(qb[:, 0:2], Bw[:, 0:6:3], wcol(1), ALU.mult)
                nc.vector.tensor_tensor(qb[:, 2:4], Bw[:, 0:6:3], wcol(2), ALU.mult)
                nc.vector.tensor_tensor(qb[:, 4:6], Bw[:, 1:6:3], wcol(2), ALU.mult)
                nc.vector.tensor_tensor(cb[:, 0:2], Cw[:, 0:6:3], wcol(1), ALU.mult)
                nc.vector.tensor_tensor(cb[:, 2:4], Cw[:, 0:6:3], wcol(2), ALU.mult)
                nc.vector.tensor_tensor(cb[:, 4:6], Cw[:, 1:6:3], wcol(2), ALU.mult)

                # Rt: [P,24], col = (4r+c)*2 + ih, c=3 holds the translation
                # Vt: [P,18], col = (3r+c)*2 + ih
                Rt = qp.tile([P, 24], F32)
                Vt = qp.tile([P, 18], F32)
                for M, st, hat, dgc, oc in ((Rt, 8, Aw, dB, qb), (Vt, 6, Bw, dC, cb)):
                    nc.vector.tensor_scalar(M[:, 0:2], dgc[:, 0:6:3], 1.0, None, ALU.add)
                    nc.vector.tensor_tensor(M[:, 2:4], oc[:, 0:2], hat[:, 2:6:3], ALU.subtract)
                    nc.vector.tensor_tensor(M[:, 4:6], oc[:, 2:4], hat[:, 1:6:3], ALU.add)
                    nc.vector.tensor_tensor(M[:, st : st + 2], oc[:, 0:2], hat[:, 2:6:3], ALU.add)
                    nc.vector.tensor_scalar(M[:, st + 2 : st + 4], dgc[:, 1:6:3], 1.0, None, ALU.add)
                    nc.vector.tensor_tensor(M[:, st + 4 : st + 6], oc[:, 4:6], hat[:, 0:6:3], ALU.subtract)
                    nc.vector.tensor_tensor(M[:, 2 * st : 2 * st + 2], oc[:, 2:4], hat[:, 1:6:3], ALU.subtract)
                    nc.vector.tensor_tensor(M[:, 2 * st + 2 : 2 * st + 4], oc[:, 4:6], hat[:, 0:6:3], ALU.add)
                    nc.vector.tensor_scalar(M[:, 2 * st + 4 : 2 * st + 6], dgc[:, 2:6:3], 1.0, None, ALU.add)

                # translation t = V @ v  into Rt cols (4r+3)*2+ih
                trall = qp.tile([P, 18], F32)   # (i, r, c)
                trv = trall[:].rearrange("p (i r c) -> p i r c", r=3, c=3)
                nc.vector.tensor_tensor(
                    trv, Vt[:].rearrange("p (r c i) -> p i r c", r=3, c=3),
                    vb.unsqueeze(2).to_broadcast((P, 2, 3, 3)), ALU.mult)
                tvb = qp.tile([P, 6], F32)      # col = r*2 + ih
                tvbv = tvb[:].rearrange("p (r i) -> p i r", r=3)
                nc.vector.tensor_reduce(tvbv, trv, AX.X, ALU.add)
                Rtv4 = Rt[:].rearrange("p (r c i) -> p i r c", r=3, c=4)
                nc.vector.tensor_copy(Rtv4[:, :, :, 3:4], tvbv.unsqueeze(3))

                # compose out = dT @ Tmat, packed [P, 32] (ih-major)
                tmib = qp.tile([P, 32], F32)
                nc.vector.tensor_copy(tmib[:, 0:16], tmi0)
                nc.vector.tensor_copy(tmib[:, 16:32], tmi1)
                Ob = qp.tile([P, 32], F32)
                prod = qp.tile([P, 32], F32)    # (i, tc, c)
                prodv = prod[:].rearrange("p (i t c) -> p i t c", t=4, c=4)
                # tmib viewed as (p, i, tc, c): col = i*16 + 4c + tc
                tmv4 = tmib[:].rearrange("p (i c t) -> p i t c", c=4, t=4)
                obv4 = Ob[:].rearrange("p (i t) -> p i t", i=2)
                for r in range(3):
                    rtv = Rtv4[:, :, r : r + 1, :].to_broadcast((P, 2, 4, 4))
                    nc.vector.tensor_tensor(prodv, tmv4, rtv, ALU.mult)
                    nc.vector.tensor_reduce(obv4[:, :, 4 * r : 4 * r + 4], prodv,
                                            AX.X, ALU.add)
                nc.vector.tensor_copy(obv4[:, :, 12:16],
                                      tmib[:].rearrange("p (i t) -> p i t", i=2)[:, :, 12:16])
                nc.sync.dma_start(out_d[:], Ob[:])

    nc.compile()
    return nc


def _q16(x):
    return np.asarray(x, np.float16).astype(np.float64)


def prep_inputs(embeddings, revisions, weights, depth, pix_T_camXs, Tmat):
    f6 = np.float64
    emb = _q16(np.asarray(embeddings, f6).reshape(B, C, N))
    rev = np.asarray(revisions, f6).reshape(B, 3, N)
    wgt = np.asarray(weights, f6).reshape(B, 3, N)
    dep = np.asarray(depth, f6).reshape(B, N)
    pix = np.asarray(pix_T_camXs, f6)
    tm = np.asarray(Tmat, f6).reshape(B, N, 16)

    ys, xs = np.meshgrid(np.arange(H, dtype=f6), np.arange(W, dtype=f6),
                         indexing="ij")
    u = xs.reshape(-1)
    v = ys.reshape(-1)

    in_maps = []
    per_batch = []
    for b in range(B):
        fx, fy, x0, y0 = pix[b, 0, 0], pix[b, 1, 1], pix[b, 0, 2], pix[b, 1, 2]
        z = _q16(dep[b])
        X = _q16((u - x0) * dep[b] / fx)
        Y = _q16((v - y0) * dep[b] / fy)
        T0 = tm[b].reshape(N, 4, 4)
        dR = _q16(T0[:, :3, :3] - np.eye(3))   # rotations are near identity
        R = np.eye(3) + dR
        t = _q16(T0[:, :3, 3])
        xyz = np.stack([X, Y, z], -1)
        TjXj = np.einsum("kpq,kq->kp", R, xyz) + t
        w0, w1, w2 = wgt[b, 0], wgt[b, 1], wgt[b, 2]
        r0, r1, r2 = rev[b, 0], rev[b, 1], rev[b, 2]
        # fold the revisions into the self-projection constants so the
        # residual is exactly fx*dX' / fy*dY' / dD' (no d / d^2 bands)
        Xkk = TjXj[:, 0] / TjXj[:, 2] + r0 / fx
        Ykk = TjXj[:, 1] / TjXj[:, 2] + r1 / fy
        dkk = 1.0 / TjXj[:, 2] + r2
        on, zn = np.ones(N), np.zeros(N)
        JT0 = np.stack([on, zn, zn, zn, -z, Y], -1)
        JT1 = np.stack([zn, on, zn, z, zn, -X], -1)
        JT2 = np.stack([zn, zn, on, -Y, X, zn], -1)
        G0 = JT0 - Xkk[:, None] * JT2
        G1 = JT1 - Ykk[:, None] * JT2

        def outer(a, bb):
            return np.einsum("kp,kq->kpq", a, bb)

        P00 = outer(G0, G0)
        P11 = outer(G1, G1)
        P22 = outer(JT2, JT2)
        P02 = outer(G0, JT2) + outer(JT2, G0)
        P12 = outer(G1, JT2) + outer(JT2, G1)
        wfx = (w0 * fx * fx)[:, None, None]
        wfy = (w1 * fy * fy)[:, None, None]
        SH = [wfx * P00 + wfy * P11, -wfx * P02, -wfy * P12,
              wfx * P22, wfy * P22, w2[:, None, None] * P22]
        SR = [fx * fx * G0,
              fy * fy * G1,
              -fx * fx * JT2,
              -fy * fy * JT2,
              -JT2]
        lam = 1.0 / (fx * fx)
        # acc stationary [128, KC*ACC_CW]; partition p of chunk c is k=c*128+p
        accst = np.zeros((P, KC * ACC_CW), f6)
        for c in range(KC):
            ks = slice(c * P, (c + 1) * P)
            co = c * ACC_CW
            for m, S in enumerate(SH):
                for ei, (p_, q_) in enumerate(HTRI):
                    accst[:, co + m * 21 + ei] = S[ks, p_, q_] * lam
            ro = co + NHB * 21
            for m, V in enumerate(SR):
                accst[:, ro + m * 6 : ro + (m + 1) * 6] = V[ks] * lam
        # geometry stationary [32, N]
        stat = np.zeros((GR, N), f6)
        stat[0:16] = -2.0 * emb[b]
        stat[16], stat[17], stat[18], stat[19] = X, Y, z, 1.0
        stat[20:24] = _q16(Xkk[None] * stat[16:20])
        stat[24:28] = _q16(Ykk[None] * stat[16:20])
        stat[28] = _q16((emb[b] ** 2).sum(0))
        stat[29] = _q16(X - Xkk * z)
        stat[30] = _q16(Y - Ykk * z)
        stat[31] = z
        per_batch.append(dict(stat=stat, accst=accst, dkk=dkk,
                              emb=emb[b], dR=dR, t=t))

    cmbH = np.zeros((P, 48), np.float32)
    for ei, (p_, q_) in enumerate(HTRI):
        cmbH[ei, p_ * 7 + q_] = 1.0
        if p_ != q_:
            cmbH[ei, q_ * 7 + p_] = 1.0
    cmbR = np.zeros((P, 48), np.float32)
    for p_ in range(6):
        cmbR[p_, p_ * 7 + 6] = 1.0

    for core in range(NCORES):
        b = core // 4
        s0 = (core % 4) * SLAB
        pb = per_batch[b]
        dRs = pb["dR"][s0 : s0 + SLAB]
        ts = pb["t"][s0 : s0 + SLAB]
        # moving operand [32, 4*SLAB]: X' | Y' | Z | s blocks
        mov = np.zeros((GR, 4 * SLAB), f6)
        for blk, row in ((0, 0), (1, 1), (2, 2)):
            mov[16:19, blk * SLAB : (blk + 1) * SLAB] = dRs[:, row, :].T
            mov[19, blk * SLAB : (blk + 1) * SLAB] = ts[:, row]
        mov[20:23, 0:SLAB] = -dRs[:, 2, :].T
        mov[23, 0:SLAB] = -ts[:, 2]
        mov[24:27, SLAB : 2 * SLAB] = -dRs[:, 2, :].T
        mov[27, SLAB : 2 * SLAB] = -ts[:, 2]
        mov[29, 0:SLAB] = 1.0
        mov[30, SLAB : 2 * SLAB] = 1.0
        mov[31, 2 * SLAB : 3 * SLAB] = 1.0
        ei_ = pb["emb"][:, s0 : s0 + SLAB]
        mov[0:16, 3 * SLAB : 4 * SLAB] = ei_
        mov[19, 3 * SLAB : 4 * SLAB] = _q16((ei_ ** 2).sum(0))
        mov[28, 3 * SLAB : 4 * SLAB] = 1.0

        geom = np.concatenate([pb["stat"], mov], 1)

        misc = np.zeros((P, 144), np.float32)
        misc[:, 0:8] = pb["dkk"].reshape(KC, P).T
        tms = np.asarray(tm[b][s0 : s0 + SLAB], np.float32)
        misc[:, 8:24] = tms[0:P]
        misc[:, 24:40] = tms[P : 2 * P]
        misc[:, 40:88] = cmbH
        misc[:, 88:136] = cmbR

        in_maps.append({
            "geom": np.ascontiguousarray(geom, np.float16),
            "accst": np.ascontiguousarray(pb["accst"], np.float16),
            "misc": np.ascontiguousarray(misc),
        })
    return in_maps


def gather_output(results):
    full = np.empty((B, N, 16), dtype=np.float32)
    for core in range(NCORES):
        b = core // 4
        s0 = (core % 4) * SLAB
        out = results[core]["out"]
        full[b, s0 : s0 + P] = out[:, 0:16]
        full[b, s0 + P : s0 + SLAB] = out[:, 16:32]
    return full.reshape(B, H, W, 4, 4)


_NC_CACHE = {}


def kernel(**inputs):
    if "nc" not in _NC_CACHE:
        _NC_CACHE["nc"] = build_nc()
    nc = _NC_CACHE["nc"]
    in_maps = prep_inputs(**inputs)
    res = run_bass_kernel_spmd(nc, in_maps, core_ids=list(range(NCORES)))
    return gather_output(res.results)
